# revision 1
# baseline (speedup 1.0000x reference)
"""Trainium2 Bass kernel for nn_EssentialMatixModule.

Dual-softmax cross-attention (LoFTR-style) + bilinear feature + projection.
Data-parallel over batch across 8 cores; proj output-sharded with chunked
AllGathers of the (bf16) feature matrix overlapping the attention phase.
"""

import sys

sys.path.insert(0, "/opt/trn_rl_repo")

from contextlib import ExitStack

import ml_dtypes
import numpy as np

import concourse.bass as bass
import concourse.tile as tile
from concourse import bacc, mybir
from concourse.bass_utils import run_bass_kernel_spmd

B, C, HG, WG = 64, 256, 24, 24
N = HG * WG  # 576
H, HD = 3, 64
F = H * HD  # 192
SCALE = HD**-0.5
EPS = 1e-5
NCORES = 8
BP = B // NCORES  # 8 items per core
NT = [128, 128, 128, 128, 64]  # token tiles (sum=576)
NCH = [(0, 512), (512, 64)]  # free-dim chunks for N=576 psum
DE = 70  # hd + 6 pos dims
PADMH = 4992  # 39*128, per-(map,head) padded feat block
DIMS = 6 * PADMH  # 29952
OS = 512 // NCORES  # 64 output cols per core
F32 = mybir.dt.float32
BF16 = mybir.dt.bfloat16
AX = mybir.ActivationFunctionType
OP = mybir.AluOpType


def _host_prep(ln_w, ln_b, qkv_w, proj_w, proj_b):
    ln_w = ln_w.astype(np.float64)
    ln_b = ln_b.astype(np.float64)
    qw = qkv_w.astype(np.float64)
    Wp = qw * ln_w[None, :]  # [576, C]
    r = Wp.sum(axis=1)  # [576]
    t = qw @ ln_b  # [576]

    # per-side packing: side0 tiles hold [k_h; q_h], side1 [q_h; k_h] so the
    # attention matmul operands always share a partition base
    def col(fsl, scale):
        c = np.zeros((C + 2,), np.float64)
        return np.concatenate([Wp[fsl] * scale, (r[fsl] * scale)[:, None],
                               (t[fsl] * scale)[:, None]], axis=1).T

    wqk = np.zeros((2, C + 2, 3 * 128), np.float32)
    for h in range(H):
        qr = slice(h * HD, (h + 1) * HD)
        kr = slice(F + h * HD, F + (h + 1) * HD)
        qcols = col(qr, SCALE)  # [C+2, 64]
        kcols = col(kr, 1.0)
        wqk[0, :, h * 128 : h * 128 + 64] = kcols
        wqk[0, :, h * 128 + 64 : h * 128 + 128] = qcols
        wqk[1, :, h * 128 : h * 128 + 64] = qcols
        wqk[1, :, h * 128 + 64 : h * 128 + 128] = kcols
    wqk = wqk.astype(ml_dtypes.bfloat16)

    wv = np.zeros((C + 2, F), np.float32)
    wv[:C] = Wp[2 * F :].T
    wv[C] = r[2 * F :]
    wv[C + 1] = t[2 * F :]
    wv = wv.astype(ml_dtypes.bfloat16)

    ys = np.linspace(-1.0, 1.0, HG)
    xs = np.linspace(-1.0, 1.0, WG)
    p3 = np.tile(ys, WG)
    p4 = np.repeat(xs, HG)
    pos = np.stack([p3 * p3, p4 * p4, p3 * p4, p3, p4, np.ones_like(p3)], axis=1)
    pos_pad = np.zeros((640, 6), np.float32)
    pos_pad[:N] = pos

    pwt = np.zeros((DIMS, 512), np.float32)
    for mh in range(6):
        blk = proj_w[:, mh * 4900 : (mh + 1) * 4900]  # [512, 4900]
        pwt[mh * PADMH : mh * PADMH + 4900] = blk.T
    pwt = pwt.astype(ml_dtypes.bfloat16)
    sel8 = np.zeros((BP, BP, 128), np.float32)
    for i in range(BP):
        sel8[i, i, :] = 1.0
    return wqk, wv, pos_pad, pwt, sel8


def _build():
    nc = bacc.Bacc()
    x1d = nc.declare_dram_parameter("x1s", [BP, C, N], BF16, isOutput=False)
    x2d = nc.declare_dram_parameter("x2s", [BP, C, N], BF16, isOutput=False)
    wqkd = nc.declare_dram_parameter("wqk", [2, C + 2, 3 * 128], BF16, isOutput=False)
    wvd = nc.declare_dram_parameter("wv", [C + 2, F], BF16, isOutput=False)
    posd = nc.declare_dram_parameter("pos", [640, 6], F32, isOutput=False)
    pwtd = nc.declare_dram_parameter("pwt", [DIMS, OS], BF16, isOutput=False)
    pbd = nc.declare_dram_parameter("pb", [1, OS], F32, isOutput=False)
    sel8d = nc.declare_dram_parameter("sel8", [BP, BP, 128], F32, isOutput=False)
    outd = nc.declare_dram_parameter("out", [B, OS], F32, isOutput=True)
    statsd = nc.dram_tensor("statsd", [2, 2, BP, N], BF16)  # (negmu, sigma)
    feat8d = [nc.dram_tensor(f"feat8_{j}", [BP, PADMH], BF16) for j in range(6)]
    featAG = [
        nc.dram_tensor(f"featAG_{j}", [B, PADMH], BF16, addr_space="Shared")
        for j in range(6)
    ]
    xd = [x1d, x2d]

    def bcast_p(sl, p):
        return bass.AP(tensor=sl.tensor, offset=sl.offset, ap=[[0, p]] + list(sl.ap))

    with ExitStack() as ctx:
        tc = ctx.enter_context(tile.TileContext(nc))
        const = ctx.enter_context(tc.tile_pool(name="const", bufs=1))
        xin = ctx.enter_context(tc.tile_pool(name="xin", bufs=6))
        stats = ctx.enter_context(tc.tile_pool(name="stats", bufs=1))
        tmp = ctx.enter_context(tc.tile_pool(name="tmp", bufs=2))
        sb_qk = ctx.enter_context(tc.tile_pool(name="sbqk", bufs=1))
        sb_vp = ctx.enter_context(tc.tile_pool(name="sbvp", bufs=1))
        epool = ctx.enter_context(tc.tile_pool(name="epool", bufs=4))
        e2pool = ctx.enter_context(tc.tile_pool(name="e2pool", bufs=6))
        zpool = ctx.enter_context(tc.tile_pool(name="zpool", bufs=12))
        upool = ctx.enter_context(tc.tile_pool(name="upool", bufs=8))
        fpool = ctx.enter_context(tc.tile_pool(name="fpool", bufs=3))
        ftpool = ctx.enter_context(tc.tile_pool(name="ftpool", bufs=3))
        opool = ctx.enter_context(tc.tile_pool(name="opool", bufs=2))
        psA = ctx.enter_context(tc.tile_pool(name="psA", bufs=2, space="PSUM"))
        psZc = ctx.enter_context(tc.tile_pool(name="psZc", bufs=1, space="PSUM"))
        psU = ctx.enter_context(tc.tile_pool(name="psU", bufs=2, space="PSUM"))

        # ---- constants ----
        wqk_sb = [[], []]
        wv_sb = []
        for k, (k0, kw) in enumerate([(0, 128), (128, 128), (256, 2)]):
            for s in range(2):
                wt = const.tile([kw, 3 * 128], BF16, tag=f"wqk{s}_{k}")
                nc.sync.dma_start(out=wt, in_=wqkd[s, k0 : k0 + kw, :])
                wqk_sb[s].append(wt)
            vt = const.tile([kw, F], BF16, tag=f"wv{k}")
            nc.sync.dma_start(out=vt, in_=wvd[k0 : k0 + kw, :])
            wv_sb.append(vt)
        pos_sb = const.tile([128, 5, 6], F32, tag="pos")
        nc.sync.dma_start(out=pos_sb, in_=posd.rearrange("(t p) e -> p t e", p=128))
        ind8_sb = const.tile([128, BP, BP], BF16, tag="ind8")
        nc.vector.memset(ind8_sb, 0.0)
        for i in range(BP):
            nc.vector.memset(ind8_sb[:, i, i : i + 1], 1.0)
        onesb_sb = const.tile([128, 32], BF16, tag="onesb")
        nc.vector.memset(onesb_sb, 1.0)
        sel8_sb = const.tile([BP, BP, 128], F32, tag="sel8")
        nc.sync.dma_start(out=sel8_sb, in_=sel8d[:])
        epssb = const.tile([32, 1], F32, tag="eps")
        nc.vector.memset(epssb, EPS)
        pb_sb = const.tile([B, OS], F32, tag="pb")
        nc.gpsimd.dma_start(out=pb_sb, in_=bcast_p(pbd[0, :], B))
        zpad = const.tile([1, 552], BF16, tag="zpad")
        nc.vector.memset(zpad, 0.0)
        for j in range(6):
            for ib in range(BP):
                nc.gpsimd.dma_start(
                    out=feat8d[j][ib, 4900:PADMH],
                    in_=bass.AP(tensor=zpad.tensor, offset=zpad.offset, ap=[[1, 1], [1, 92]]),
                )

        # ---- phase 1a: LN stats ----
        isv_sb = []
        isColT = []
        for s in range(2):
            psum_s = psA.tile([BP, N], F32, tag="pA")
            psum_q = psA.tile([BP, N], F32, tag="pA")
            for i in range(BP):
                for k in range(2):
                    xt = xin.tile([128, N], BF16, tag="x")
                    nc.scalar.dma_start(out=xt, in_=xd[s][i, k * 128 : (k + 1) * 128, :])
                    xq = tmp.tile([128, N], BF16, tag="xsq")
                    nc.vector.tensor_mul(xq, xt, xt)
                    st = i == 0 and k == 0
                    for c0, cw in NCH:
                        nc.tensor.matmul(
                            psum_s[:, c0 : c0 + cw], ind8_sb[:, i, :], xt[:, c0 : c0 + cw],
                            start=st, stop=(i == BP - 1 and k == 1),
                        )
                        nc.tensor.matmul(
                            psum_q[:, c0 : c0 + cw], ind8_sb[:, i, :], xq[:, c0 : c0 + cw],
                            start=st, stop=(i == BP - 1 and k == 1),
                        )
            mean = stats.tile([32, N], F32, tag="mean")
            ex2 = stats.tile([32, N], F32, tag="ex2")
            nc.vector.tensor_scalar_mul(mean[:BP], psum_s, 1.0 / C)
            nc.vector.tensor_scalar_mul(ex2[:BP], psum_q, 1.0 / C)
            var = stats.tile([32, N], F32, tag="var")
            nc.vector.scalar_tensor_tensor(
                out=var[:BP], in0=mean[:BP], scalar=-1.0, in1=mean[:BP], op0=OP.mult, op1=OP.mult
            )
            nc.vector.tensor_add(var[:BP], var[:BP], ex2[:BP])
            sig = stats.tile([32, N], F32, tag="sig")
            nc.scalar.activation(out=sig[:BP], in_=var[:BP], func=AX.Sqrt, bias=epssb[:BP])
            isv = stats.tile([32, N], F32, tag=f"isv{s}")
            nc.vector.reciprocal(isv[:BP], sig[:BP])
            negmu = stats.tile([32, N], BF16, tag="negmu")
            nc.vector.tensor_scalar_mul(negmu[:BP], mean[:BP], -1.0)
            sigb = stats.tile([32, N], BF16, tag="sigb")
            nc.vector.tensor_copy(sigb[:BP], sig[:BP])
            nc.sync.dma_start(out=statsd[s, 0], in_=negmu[:BP])
            nc.sync.dma_start(out=statsd[s, 1], in_=sigb[:BP])
            zt_is = stats.tile([32, 18, 32], F32, tag="zt_is")
            nc.vector.transpose(out=zt_is, in_=isv.rearrange("p (g q) -> p g q", q=32))
            ict = const.tile([128, 5, BP], F32, tag=f"iscol{s}")
            for a in range(4):
                ng = 5 if a < 2 else 4
                nc.vector.tensor_copy(
                    out=ict[32 * a : 32 * a + 32, 0:ng, :],
                    in_=zt_is[:, a : 18 : 4, 0:BP],
                )
            isv_sb.append(isv)
            isColT.append(ict)

        # ---- phase 1b: QKV for all items ----
        qs = {}
        ks = {}
        vp = {}
        for i in range(BP):
            for s in range(2):
                xe = tmp.tile([2, N], BF16, tag="xe")
                nc.sync.dma_start(out=xe, in_=statsd[s, :, i, :])
                # broadcast 1/sigma row across 128 partitions via K=1 matmul
                pis = psA.tile([128, N], F32, tag="pA")
                for c0, cw in NCH:
                    nc.tensor.matmul(
                        pis[:, c0 : c0 + cw], sel8_sb[:, i, :], isv_sb[s][0:BP, c0 : c0 + cw],
                        start=True, stop=True,
                    )
                isb = tmp.tile([128, N], F32, tag="isb")
                nc.vector.tensor_copy(out=isb, in_=pis)
                xt0 = xin.tile([128, N], BF16, tag="x")
                nc.scalar.dma_start(out=xt0, in_=xd[s][i, 0:128, :])
                xt1 = xin.tile([128, N], BF16, tag="x")
                nc.scalar.dma_start(out=xt1, in_=xd[s][i, 128:256, :])
                rhs3 = [xt0, xt1, xe]
                # per-side packed [k;q] (s=0) / [q;k] (s=1) tiles: one full-width
                # scale op per head, and A-matmul operands share partition bases
                for h in range(H):
                    pq = psA.tile([128, N], F32, tag="pA")
                    for k in range(3):
                        for c0, cw in NCH:
                            nc.tensor.matmul(
                                pq[:, c0 : c0 + cw],
                                wqk_sb[s][k][:, h * 128 : (h + 1) * 128],
                                rhs3[k][:, c0 : c0 + cw],
                                start=(k == 0), stop=(k == 2),
                            )
                    qk = sb_qk.tile([128, N], BF16, tag=f"qk{i}_{s}_{h}")
                    nc.vector.tensor_mul(qk, pq, isb)
                    if s == 0:
                        ks[i, s, h] = qk[0:64, :]
                        qs[i, s, h] = qk[64:128, :]
                    else:
                        qs[i, s, h] = qk[0:64, :]
                        ks[i, s, h] = qk[64:128, :]
                for nt in range(5):
                    w = NT[nt]
                    n0 = nt * 128
                    pv = psA.tile([128, F], F32, tag="pA")
                    for k in range(3):
                        nc.tensor.matmul(
                            pv[:w],
                            rhs3[k][:, n0 : n0 + w],
                            wv_sb[k],
                            start=(k == 0), stop=(k == 2),
                        )
                    vt = sb_vp.tile([128, 3, 72], BF16, tag=f"vp{i}_{s}_{nt}")
                    nc.vector.tensor_scalar_mul(
                        vt[:w, :, 0:64],
                        pv[:w, 0:F].rearrange("p (a b) -> p a b", b=64),
                        isColT[s][:w, nt, i : i + 1],
                    )
                    ps = pos_sb[:w, nt, :]
                    nc.vector.tensor_copy(
                        out=vt[:w, :, 64:70],
                        in_=bass.AP(tensor=ps.tensor, offset=ps.offset,
                                    ap=[ps.ap[0], [0, 3], ps.ap[-1]]),
                    )
                    vp[i, s, nt] = vt

        # ---- phase 2: attention, mh-outer; chunked AllGather + proj overlap ----
        oacc = opool.tile([B, OS], F32, tag="oacc")
        nc.vector.memset(oacc, 0.0)

        def emit_proj(mh):
            # runs one mh-block of the projection; called one block late so the
            # AllGather it reads has completed and never head-of-line blocks
            GSZ = 13
            for g0 in range(0, 39, GSZ):
                ft = ftpool.tile([128, GSZ, B], BF16, tag="ft")
                nc.sync.dma_start_transpose(
                    out=ft, in_=featAG[mh][:, g0 * 128 : (g0 + GSZ) * 128]
                )
                pw = ftpool.tile([128, GSZ, OS], BF16, tag="pw")
                nc.scalar.dma_start(
                    out=pw,
                    in_=pwtd[mh * PADMH + g0 * 128 : mh * PADMH + (g0 + GSZ) * 128, :]
                    .rearrange("(j p) o -> p j o", p=128),
                )
                opsum = psU.tile([64, OS], F32, tag="pU")
                for j in range(GSZ):
                    nc.tensor.matmul(
                        opsum, ft[:, j, :], pw[:, j, :],
                        start=(j == 0), stop=(j == GSZ - 1),
                    )
                nc.vector.tensor_add(oacc, oacc, opsum)

        def emit_gather(j):
            nc.gpsimd.collective_compute(
                "AllGather",
                OP.bypass,
                replica_groups=[list(range(NCORES))],
                ins=[feat8d[j][:]],
                outs=[featAG[j][:]],
            )

        # proj blocks emitted only when their gather has had a full attention
        # block to complete, keeping DMA-transposes clear of collectives
        proj_at = {3: [0, 1], 5: [2, 3]}
        gather_at = {1: [0, 1], 3: [2, 3], 4: [4]}
        for m in range(2):
            qside = 1 - m
            vside = m
            for h in range(H):
                mh = m * 3 + h
                for pj in proj_at.get(mh, []):
                    emit_proj(pj)
                for i in range(BP):
                    zcp = psZc.tile([32, N], F32, tag="pZc")
                    zr5 = zpool.tile([128, 8], F32, tag="zr5")
                    nc.vector.memset(zr5, 1.0)
                    e2s = []
                    for nt in range(5):
                        w = NT[nt]
                        n0 = nt * 128
                        pa = psA.tile([128, N], F32, tag="pA")
                        for c0, cw in NCH:
                            nc.tensor.matmul(
                                pa[:w, c0 : c0 + cw],
                                qs[i, qside, h][:, n0 : n0 + w],
                                ks[i, vside, h][:, c0 : c0 + cw],
                                start=True, stop=True,
                            )
                        et = epool.tile([128, N], BF16, tag="E")
                        nc.scalar.activation(
                            out=et[:w], in_=pa[:w], func=AX.Exp,
                            accum_out=zr5[:w, nt : nt + 1],
                        )
                        for c0, cw in NCH:
                            nc.tensor.matmul(
                                zcp[:, c0 : c0 + cw], onesb_sb[:w, :], et[:w, c0 : c0 + cw],
                                start=(nt == 0), stop=(nt == 4),
                            )
                        e2 = e2pool.tile([128, N], BF16, tag="E2")
                        nc.vector.tensor_mul(e2[:w], et[:w], et[:w])
                        e2s.append(e2)
                    zt = tmp.tile([32, 18, 32], F32, tag="zt")
                    nc.vector.transpose(
                        out=zt, in_=zcp.rearrange("p (g q) -> p g q", q=32)
                    )
                    rz32 = zpool.tile([32, 18], F32, tag="rz32")
                    nc.vector.reciprocal(rz32, zt[:, :, 0])
                    rzc = zpool.tile([128, 5], F32, tag="rzc")
                    for a in range(4):
                        ng = 5 if a < 2 else 4
                        nc.vector.tensor_copy(
                            out=rzc[32 * a : 32 * a + 32, 0:ng],
                            in_=rz32[:, a : 18 : 4],
                        )
                    # 1/Zr row-scale lives on small vp copies, not on E^2
                    rzr5 = zpool.tile([128, 8], F32, tag="rzr5")
                    nc.vector.reciprocal(rzr5[:, 0:5], zr5[:, 0:5])
                    vpls = []
                    for nt in range(5):
                        w = NT[nt]
                        vpl = upool.tile([128, 72], BF16, tag="vpl")
                        nc.vector.tensor_scalar_mul(
                            vpl[:w, 0:70], vp[i, vside, nt][:w, h, 0:70],
                            rzr5[:w, nt : nt + 1],
                        )
                        vpls.append(vpl)
                    fps = psU.tile([128, 72], F32, tag="pU")
                    for mc in range(5):
                        w2 = NT[mc]
                        up = psU.tile([128, 72], F32, tag="pU")
                        for nt in range(5):
                            w = NT[nt]
                            nc.tensor.matmul(
                                up[:w2, 0:70],
                                e2s[nt][:w, mc * 128 : mc * 128 + w2],
                                vpls[nt][:w, 0:70],
                                start=(nt == 0), stop=(nt == 4),
                            )
                        us = upool.tile([128, 72], BF16, tag="us")
                        nc.vector.tensor_scalar_mul(
                            us[:w2, 0:70], up[:w2, 0:70], rzc[:w2, mc : mc + 1]
                        )
                        nc.tensor.matmul(
                            fps[0:70, 0:70],
                            us[:w2, 0:70],
                            vp[i, vside, mc][:w2, h, 0:70],
                            start=(mc == 0), stop=(mc == 4),
                        )
                    fb = fpool.tile([70, 70], BF16, tag="fb")
                    nc.vector.tensor_copy(out=fb, in_=fps[0:70, 0:70])
                    nc.sync.dma_start(
                        out=feat8d[mh][i, 0:4900].rearrange("(d e) -> d e", e=70),
                        in_=fb,
                    )
                for j in gather_at.get(mh, []):
                    emit_gather(j)

        emit_proj(4)
        emit_gather(5)
        emit_proj(5)
        osb = opool.tile([B, OS], F32, tag="osb")
        nc.vector.tensor_add(osb, oacc, pb_sb)
        nc.vector.tensor_scalar_max(osb, osb, 0.0)
        nc.sync.dma_start(out=outd[:], in_=osb)

    nc.compile()
    return nc


def kernel(x1, x2, ln_w, ln_b, qkv_w, proj_w, proj_b):
    wqk, wv, pos_pad, pwt, sel8 = _host_prep(ln_w, ln_b, qkv_w, proj_w, proj_b)
    xs1 = np.ascontiguousarray(x1.reshape(B, C, N)).astype(ml_dtypes.bfloat16)
    xs2 = np.ascontiguousarray(x2.reshape(B, C, N)).astype(ml_dtypes.bfloat16)
    nc = _build()
    in_maps = []
    for r in range(NCORES):
        in_maps.append(
            {
                "x1s": xs1[r * BP : (r + 1) * BP],
                "x2s": xs2[r * BP : (r + 1) * BP],
                "wqk": wqk,
                "wv": wv,
                "pos": pos_pad,
                "pwt": np.ascontiguousarray(pwt[:, r * OS : (r + 1) * OS]),
                "pb": np.ascontiguousarray(proj_b[None, r * OS : (r + 1) * OS]).astype(np.float32),
                "sel8": sel8,
            }
        )
    import os
    import time as _time

    trace = bool(os.environ.get("BASS_TRACE"))
    res = run_bass_kernel_spmd(nc, in_maps, core_ids=list(range(NCORES)), trace=trace)
    nruns = int(os.environ.get("PROF_RUNS", "0"))
    if nruns:
        times = []
        for _ in range(nruns):
            t0 = _time.perf_counter()
            run_bass_kernel_spmd(nc, in_maps, core_ids=list(range(NCORES)))
            times.append(_time.perf_counter() - t0)
        print(f"exec wall times (s): {[f'{t:.3f}' for t in times]}, min={min(times)*1e6:.0f} us")
    if res.exec_time_ns is not None:
        print(f"HW exec time: {res.exec_time_ns} ns")
    if res.instructions_and_trace:
        print("trace path:", res.instructions_and_trace[1])
    out = np.concatenate([res.results[r]["out"] for r in range(NCORES)], axis=1)
    return out.astype(np.float32)


if __name__ == "__main__":
    rng = np.random.default_rng(0)
    ins = {
        "x1": rng.standard_normal((B, C, HG, WG), dtype=np.float32),
        "x2": rng.standard_normal((B, C, HG, WG), dtype=np.float32),
        "ln_w": np.ones(C, np.float32),
        "ln_b": np.zeros(C, np.float32),
        "qkv_w": (rng.standard_normal((3 * F, C)) * C**-0.5).astype(np.float32),
        "proj_w": (rng.standard_normal((512, 6 * 4900)) * (6 * 4900) ** -0.5).astype(np.float32),
        "proj_b": np.zeros(512, np.float32),
    }
    print(kernel(**ins).shape)



# revision 27
# speedup vs baseline: 1.1661x; 1.1661x over previous
"""Trainium2 Bass kernel for nn_EssentialMatixModule.

Dual-softmax cross-attention (LoFTR-style) + bilinear feature + projection.
Data-parallel over batch across 8 cores; proj output-sharded with chunked
AllGathers of the (bf16) feature matrix overlapping the attention phase.

v2: 4-deep software-pipelined attention blocks (QK -> exp/Zsum -> DVE chain
-> U/F) so every engine queue streams dependency-free work; 1/Zr folded into
the E^2 squares, 1/Zc folded into the single U->SBUF copy; 1/sigma broadcast
via DMA instead of fp32 matmuls; split tail AllGather.
"""

import sys

sys.path.insert(0, "/opt/trn_rl_repo")

from contextlib import ExitStack

import ml_dtypes
import numpy as np

import concourse.bass as bass
import concourse.tile as tile
from concourse import bacc, mybir
from concourse.bass_utils import run_bass_kernel_spmd

B, C, HG, WG = 64, 256, 24, 24
N = HG * WG  # 576
H, HD = 3, 64
F = H * HD  # 192
SCALE = HD**-0.5
EPS = 1e-5
NCORES = 8
BP = B // NCORES  # 8 items per core
NT = [128, 128, 128, 128, 64]  # token tiles (sum=576)
NCH = [(0, 512), (512, 64)]  # free-dim chunks for N=576 psum
DE = 70  # hd + 6 pos dims
PADMH = 4992  # 39*128, per-(map,head) padded feat block
OS = 512 // NCORES  # 64 output cols per core
F32 = mybir.dt.float32
BF16 = mybir.dt.bfloat16
AX = mybir.ActivationFunctionType
OP = mybir.AluOpType


def _host_prep(ln_w, ln_b, qkv_w, proj_w, proj_b):
    ln_w = ln_w.astype(np.float64)
    ln_b = ln_b.astype(np.float64)
    qw = qkv_w.astype(np.float64)
    Wp = qw * ln_w[None, :]  # [576, C]
    r = Wp.sum(axis=1)  # [576]
    t = qw @ ln_b  # [576]

    # per-side packing: side0 tiles hold [k_h; q_h], side1 [q_h; k_h] so the
    # attention matmul operands always share a partition base
    def col(fsl, scale):
        return np.concatenate([Wp[fsl] * scale, (r[fsl] * scale)[:, None],
                               (t[fsl] * scale)[:, None]], axis=1).T

    wqk = np.zeros((2, C + 2, 3 * 128), np.float32)
    for h in range(H):
        qr = slice(h * HD, (h + 1) * HD)
        kr = slice(F + h * HD, F + (h + 1) * HD)
        qcols = col(qr, SCALE)  # [C+2, 64]
        kcols = col(kr, 1.0)
        wqk[0, :, h * 128 : h * 128 + 64] = kcols
        wqk[0, :, h * 128 + 64 : h * 128 + 128] = qcols
        wqk[1, :, h * 128 : h * 128 + 64] = qcols
        wqk[1, :, h * 128 + 64 : h * 128 + 128] = kcols
    wqk = wqk.astype(ml_dtypes.bfloat16)

    wv = np.zeros((C + 2, F), np.float32)
    wv[:C] = Wp[2 * F :].T
    wv[C] = r[2 * F :]
    wv[C + 1] = t[2 * F :]
    wv = wv.astype(ml_dtypes.bfloat16)

    ys = np.linspace(-1.0, 1.0, HG)
    xs = np.linspace(-1.0, 1.0, WG)
    p3 = np.tile(ys, WG)
    p4 = np.repeat(xs, HG)
    pos = np.stack([p3 * p3, p4 * p4, p3 * p4, p3, p4, np.ones_like(p3)], axis=1)
    pos_pad = np.zeros((640, 6), np.float32)
    pos_pad[:N] = pos

    DIMS = 6 * PADMH
    pwt = np.zeros((DIMS, 512), np.float32)
    for mh in range(6):
        blk = proj_w[:, mh * 4900 : (mh + 1) * 4900]  # [512, 4900]
        pwt[mh * PADMH : mh * PADMH + 4900] = blk.T
    pwt = pwt.astype(ml_dtypes.bfloat16)
    return wqk, wv, pos_pad, pwt


def _build():
    nc = bacc.Bacc()
    DIMS = 6 * PADMH
    x1d = nc.declare_dram_parameter("x1s", [BP, C, N], BF16, isOutput=False)
    x2d = nc.declare_dram_parameter("x2s", [BP, C, N], BF16, isOutput=False)
    wqkd = nc.declare_dram_parameter("wqk", [2, C + 2, 3 * 128], BF16, isOutput=False)
    wvd = nc.declare_dram_parameter("wv", [C + 2, F], BF16, isOutput=False)
    posd = nc.declare_dram_parameter("pos", [640, 6], F32, isOutput=False)
    pwtd = nc.declare_dram_parameter("pwt", [DIMS, OS], BF16, isOutput=False)
    pbd = nc.declare_dram_parameter("pb", [1, OS], F32, isOutput=False)
    outd = nc.declare_dram_parameter("out", [B, OS], F32, isOutput=True)
    # feature chunks: mh 0..4 full, mh 5 split by item halves for the tail
    feat8d = [nc.dram_tensor(f"feat8_{j}", [BP, PADMH], BF16) for j in range(5)]
    feat5a = nc.dram_tensor("feat5a", [BP // 2, PADMH], BF16)
    feat5b = nc.dram_tensor("feat5b", [BP // 2, PADMH], BF16)
    featAG = [
        nc.dram_tensor(f"featAG_{j}", [B, PADMH], BF16, addr_space="Shared")
        for j in range(5)
    ]
    featAG5a = nc.dram_tensor("featAG5a", [B // 2, PADMH], BF16, addr_space="Shared")
    featAG5b = nc.dram_tensor("featAG5b", [B // 2, PADMH], BF16, addr_space="Shared")
    isvd = nc.dram_tensor("isvd", [2, BP, N], F32)
    xd = [x1d, x2d]

    def bcast_p(sl, p):
        return bass.AP(tensor=sl.tensor, offset=sl.offset, ap=[[0, p]] + list(sl.ap))

    def bcast_f(sl, n):
        # broadcast along a new innermost free dim of size n
        return bass.AP(tensor=sl.tensor, offset=sl.offset, ap=list(sl.ap) + [[0, n]])

    with ExitStack() as ctx:
        tc = ctx.enter_context(tile.TileContext(nc))
        const = ctx.enter_context(tc.tile_pool(name="const", bufs=1))
        stats = ctx.enter_context(tc.tile_pool(name="stats", bufs=1))
        tmp = ctx.enter_context(tc.tile_pool(name="tmp", bufs=2))
        xin = ctx.enter_context(tc.tile_pool(name="xin", bufs=6))
        isbp = ctx.enter_context(tc.tile_pool(name="isbp", bufs=4))
        sb_qk = ctx.enter_context(tc.tile_pool(name="sbqk", bufs=1))
        sb_vp = ctx.enter_context(tc.tile_pool(name="sbvp", bufs=1))
        epool = ctx.enter_context(tc.tile_pool(name="epool", bufs=11))
        e2pool = ctx.enter_context(tc.tile_pool(name="e2pool", bufs=11))
        zpool = ctx.enter_context(tc.tile_pool(name="zpool", bufs=3))
        uspool = ctx.enter_context(tc.tile_pool(name="uspool", bufs=2))
        fpool = ctx.enter_context(tc.tile_pool(name="fpool", bufs=2))
        ftpool = ctx.enter_context(tc.tile_pool(name="ftpool", bufs=2))
        opool = ctx.enter_context(tc.tile_pool(name="opool", bufs=1))
        psA = ctx.enter_context(tc.tile_pool(name="psA", bufs=2, space="PSUM"))
        psZ = ctx.enter_context(tc.tile_pool(name="psZ", bufs=1, space="PSUM"))
        psU = ctx.enter_context(tc.tile_pool(name="psU", bufs=2, space="PSUM"))

        # ---- constants ----
        wqk_sb = [[], []]
        wv_sb = []
        for k, (k0, kw) in enumerate([(0, 128), (128, 128), (256, 2)]):
            for s in range(2):
                wt = const.tile([kw, 3 * 128], BF16, tag=f"wqk{s}_{k}")
                nc.sync.dma_start(out=wt, in_=wqkd[s, k0 : k0 + kw, :])
                wqk_sb[s].append(wt)
            vt = const.tile([kw, F], BF16, tag=f"wv{k}")
            nc.sync.dma_start(out=vt, in_=wvd[k0 : k0 + kw, :])
            wv_sb.append(vt)
        pos_sb = const.tile([128, 5, 6], F32, tag="pos")
        nc.sync.dma_start(out=pos_sb, in_=posd.rearrange("(t p) e -> p t e", p=128))
        ind8_sb = const.tile([128, BP, BP], BF16, tag="ind8")
        nc.vector.memset(ind8_sb, 0.0)
        for i in range(BP):
            nc.vector.memset(ind8_sb[:, i, i : i + 1], 1.0)
        onesb_sb = const.tile([128, 32], BF16, tag="onesb")
        nc.vector.memset(onesb_sb, 1.0)
        epssb = const.tile([32, 1], F32, tag="eps")
        nc.vector.memset(epssb, EPS)
        pb_sb = const.tile([B, OS], F32, tag="pb")
        nc.gpsimd.dma_start(out=pb_sb, in_=bcast_p(pbd[0, :], B))
        oacc = const.tile([B, OS], F32, tag="oacc")
        nc.vector.memset(oacc, 0.0)
        zft = const.tile([128, OS], BF16, tag="zft")
        nc.vector.memset(zft, 0.0)

        # ---- phase 1a: LN stats ----
        isv_sb = []
        isColT = []
        xe_all = []
        for s in range(2):
            psum_s = psA.tile([128, N], F32, tag="A")
            psum_q = psA.tile([128, N], F32, tag="A")
            for i in range(BP):
                for k in range(2):
                    xt = xin.tile([128, N], BF16, tag="x")
                    nc.sync.dma_start(out=xt, in_=xd[s][i, k * 128 : (k + 1) * 128, :])
                    xq = tmp.tile([128, N], BF16, tag="xsq")
                    if k == 0:
                        nc.vector.tensor_mul(xq, xt, xt)
                    else:
                        nc.gpsimd.tensor_mul(xq, xt, xt)
                    st = i == 0 and k == 0
                    for c0, cw in NCH:
                        nc.tensor.matmul(
                            psum_s[0:BP, c0 : c0 + cw], ind8_sb[:, i, :], xt[:, c0 : c0 + cw],
                            start=st, stop=(i == BP - 1 and k == 1),
                        )
                        nc.tensor.matmul(
                            psum_q[0:BP, c0 : c0 + cw], ind8_sb[:, i, :], xq[:, c0 : c0 + cw],
                            start=st, stop=(i == BP - 1 and k == 1),
                        )
            mean = stats.tile([32, N], F32, tag="mean")
            ex2 = stats.tile([32, N], F32, tag="ex2")
            nc.vector.tensor_scalar_mul(mean[:BP], psum_s[0:BP], 1.0 / C)
            nc.vector.tensor_scalar_mul(ex2[:BP], psum_q[0:BP], 1.0 / C)
            var = stats.tile([32, N], F32, tag="var")
            nc.vector.scalar_tensor_tensor(
                out=var[:BP], in0=mean[:BP], scalar=-1.0, in1=mean[:BP], op0=OP.mult, op1=OP.mult
            )
            nc.vector.tensor_add(var[:BP], var[:BP], ex2[:BP])
            sig = stats.tile([32, N], F32, tag="sig")
            nc.scalar.activation(out=sig[:BP], in_=var[:BP], func=AX.Sqrt, bias=epssb[:BP])
            isv = stats.tile([32, N], F32, tag=f"isv{s}")
            nc.vector.reciprocal(isv[:BP], sig[:BP])
            negmu = stats.tile([32, N], BF16, tag="negmu")
            nc.vector.tensor_scalar_mul(negmu[:BP], mean[:BP], -1.0)
            sigb = stats.tile([32, N], BF16, tag="sigb")
            nc.vector.tensor_copy(sigb[:BP], sig[:BP])
            # xe_all[s][0] = negmu rows, [1] = sigma rows  (SBUF->SBUF DMA)
            xe = stats.tile([2, BP, N], BF16, tag=f"xe{s}")
            nc.gpsimd.dma_start(out=xe[0:1, :, :], in_=negmu[0:BP, :])
            nc.gpsimd.dma_start(out=xe[1:2, :, :], in_=sigb[0:BP, :])
            xe_all.append(xe)
            nc.gpsimd.dma_start(out=isvd[s], in_=isv[0:BP, :])
            zt_is = stats.tile([32, 18, 32], F32, tag="zt_is")
            nc.vector.transpose(out=zt_is, in_=isv.rearrange("p (g q) -> p g q", q=32))
            ict = const.tile([128, 5, BP], F32, tag=f"iscol{s}")
            for a in range(4):
                ng = 5 if a < 2 else 4
                nc.vector.tensor_copy(
                    out=ict[32 * a : 32 * a + 32, 0:ng, :],
                    in_=zt_is[:, a : 18 : 4, 0:BP],
                )
            isv_sb.append(isv)
            isColT.append(ict)

        # ---- phase 1b: QKV for all items ----
        qs = {}
        ks = {}
        vp = {}
        isb_t = {}
        for i in range(BP):
            for s in range(2):
                # broadcast 1/sigma row across 128 partitions via DMA
                isb = isbp.tile([128, N], F32, tag="isb")
                nc.gpsimd.dma_start(out=isb, in_=bcast_p(isvd[s, i, :], 128))
                isb_t[i, s] = isb
        for i in range(BP):
            for s in range(2):
                xt0 = xin.tile([128, N], BF16, tag="x")
                nc.sync.dma_start(out=xt0, in_=xd[s][i, 0:128, :])
                xt1 = xin.tile([128, N], BF16, tag="x")
                nc.sync.dma_start(out=xt1, in_=xd[s][i, 128:256, :])
                rhs3 = [xt0, xt1, xe_all[s][:, i, :]]
                # per-side packed [k;q] (s=0) / [q;k] (s=1) tiles: one full-width
                # scale op per head, and A-matmul operands share partition bases
                for h in range(H):
                    pq = psA.tile([128, N], F32, tag="A")
                    for k in range(3):
                        for c0, cw in NCH:
                            nc.tensor.matmul(
                                pq[:, c0 : c0 + cw],
                                wqk_sb[s][k][:, h * 128 : (h + 1) * 128],
                                rhs3[k][:, c0 : c0 + cw],
                                start=(k == 0), stop=(k == 2),
                            )
                    qk = sb_qk.tile([128, N], BF16, tag=f"qk{i}_{s}_{h}")
                    nc.vector.tensor_mul(qk, pq, isb_t[i, s])
                    if s == 0:
                        ks[i, s, h] = qk[0:64, :]
                        qs[i, s, h] = qk[64:128, :]
                    else:
                        qs[i, s, h] = qk[0:64, :]
                        ks[i, s, h] = qk[64:128, :]
                for nt in range(5):
                    w = NT[nt]
                    n0 = nt * 128
                    pv = psA.tile([128, N], F32, tag="A")
                    for k in range(3):
                        nc.tensor.matmul(
                            pv[:w, 0:F],
                            rhs3[k][:, n0 : n0 + w],
                            wv_sb[k],
                            start=(k == 0), stop=(k == 2),
                        )
                    vt = sb_vp.tile([128, 3, 72], BF16, tag=f"vp{i}_{s}_{nt}")
                    # v scale by 1/sigma (per-token partition scalar) on ACT
                    nc.scalar.activation(
                        out=vt[:w, :, 0:64],
                        in_=pv[:w, 0:F].rearrange("p (a b) -> p a b", b=64),
                        func=AX.Copy,
                        scale=isColT[s][:w, nt, i : i + 1],
                    )
                    ps = pos_sb[:w, nt, :]
                    nc.gpsimd.tensor_copy(
                        out=vt[:w, :, 64:70],
                        in_=bass.AP(tensor=ps.tensor, offset=ps.offset,
                                    ap=[ps.ap[0], [0, 3], ps.ap[-1]]),
                    )
                    vp[i, s, nt] = vt

        # ---- phase 2: 4-deep pipelined attention; chunked AllGather + proj ----
        blocks = []
        for m in range(2):
            for h in range(H):
                for i in range(BP):
                    blocks.append((m, h, i))
        NB = len(blocks)  # 48

        # per-block state carried between pipeline stages
        A_t = {}
        et_t = {}
        zr5_t = {}
        rzr5_t = {}
        zcp_t = {}
        e2_t = {}
        rzc_t = {}
        us_t = {}

        def emit_qk_tile(g):
            # one nt-tile of a block's attention scores; A psum rotation = 2
            b, nt = divmod(g, 5)
            m, h, i = blocks[b]
            qside, vside = 1 - m, m
            if nt == 0:
                # zr5 memset early, off the ACT critical path
                zr5 = zpool.tile([128, 8], F32, tag="zr5")
                nc.gpsimd.memset(zr5, 1.0)
                zr5_t[b] = zr5
                et_t[b] = []
                zcp_t[b] = psZ.tile([32, N], F32, tag="zc", name="zcp")
            w = NT[nt]
            n0 = nt * 128
            pa = psA.tile([128, N], F32, tag="A")
            for c0, cw in NCH:
                nc.tensor.matmul(
                    pa[:w, c0 : c0 + cw],
                    qs[i, qside, h][:, n0 : n0 + w],
                    ks[i, vside, h][:, c0 : c0 + cw],
                    start=True, stop=True,
                )
            A_t[g] = pa

        def emit_exp_tile(g):
            b, nt = divmod(g, 5)
            pa = A_t.pop(g)
            w = NT[nt]
            et = epool.tile([128, N], BF16, tag="E")
            nc.scalar.activation(
                out=et[:w], in_=pa[:w, :], func=AX.Exp,
                accum_out=zr5_t[b][:w, nt : nt + 1],
            )
            et_t[b].append(et)

        def emit_zsum_tile(g):
            b, nt = divmod(g, 5)
            w = NT[nt]
            et = et_t[b][nt]
            for c0, cw in NCH:
                nc.tensor.matmul(
                    zcp_t[b][:, c0 : c0 + cw], onesb_sb[:w, :], et[:w, c0 : c0 + cw],
                    start=(nt == 0), stop=(nt == 4),
                )

        def emit_dve_chain(b):
            # all inputs were produced last step: no DVE head-of-line stalls
            ets = et_t.pop(b)
            zr5 = zr5_t.pop(b)
            zcp = zcp_t.pop(b)
            rzr5 = zpool.tile([128, 8], F32, tag="rzr5")
            nc.vector.reciprocal(rzr5, zr5)
            e2s = []
            vls = {}
            m_, h_, i_ = blocks[b]
            for nt in range(5):
                w = NT[nt]
                e2 = e2pool.tile([128, N], BF16, tag="E2")
                if nt < 3:
                    # e2 = (E * (1/Zr)_row) * E  -- folds the row softmax scale
                    nc.vector.scalar_tensor_tensor(
                        out=e2[:w], in0=ets[nt][:w], scalar=rzr5[:w, nt : nt + 1],
                        in1=ets[nt][:w], op0=OP.mult, op1=OP.mult,
                    )
                else:
                    # Pool can't run TensorScalarPtr: plain square there, and
                    # fold 1/Zr into a small scaled-vp copy on DVE instead
                    nc.gpsimd.tensor_mul(e2[:w], ets[nt][:w], ets[nt][:w])
                    vl = uspool.tile([128, 72], BF16, tag=f"vl{nt}")
                    nc.vector.tensor_scalar_mul(
                        vl[:w, 0:70], vp[i_, m_, nt][:w, h_, 0:70],
                        rzr5[:w, nt : nt + 1],
                    )
                    vls[nt] = vl
                e2s.append(e2)
            zt = tmp.tile([32, 18, 32], F32, tag="zt")
            nc.vector.transpose(out=zt, in_=zcp.rearrange("p (g q) -> p g q", q=32))
            rz32 = zpool.tile([32, 18], F32, tag="rz32")
            nc.vector.reciprocal(rz32, zt[:, :, 0])
            rzc = zpool.tile([128, 8], F32, tag="rzc")
            for a in range(4):
                ng = 5 if a < 2 else 4
                nc.vector.tensor_copy(
                    out=rzc[32 * a : 32 * a + 32, 0:ng],
                    in_=rz32[:, a : 18 : 4],
                )
            e2_t[b] = e2s
            rzc_t[b] = rzc
            vl_t[b] = vls

        up_t = {}
        vl_t = {}

        def emit_u_group(b, mc):
            m, h, i = blocks[b]
            vside = m
            e2s = e2_t[b]
            if mc == 0:
                up_t[b] = psU.tile([128, 5, 70], F32, tag="pU", name="up")
            up = up_t[b]
            w2 = NT[mc]
            for nt in range(5):
                w = NT[nt]
                rhs = (vp[i, vside, nt][:w, h, 0:70] if nt < 3
                       else vl_t[b][nt][:w, 0:70])
                nc.tensor.matmul(
                    up[:w2, mc, :],
                    e2s[nt][:w, mc * 128 : mc * 128 + w2],
                    rhs,
                    start=(nt == 0), stop=(nt == 4),
                )

        def emit_us(b):
            # one copy applies the col-softmax scale: us[p,mc,:] = up * rzc[p,mc]
            e2_t.pop(b)
            vl_t.pop(b)
            up = up_t.pop(b)
            rzc = rzc_t.pop(b)
            us = uspool.tile([128, 5, 70], BF16, tag="us")
            nc.vector.tensor_tensor(
                out=us, in0=up,
                in1=bass.AP(tensor=rzc.tensor, offset=rzc.offset,
                            ap=[rzc.ap[0], [1, 5], [0, 70]]),
                op=OP.mult,
            )
            us_t[b] = us

        def emit_f(b):
            m, h, i = blocks[b]
            vside = m
            mh = m * 3 + h
            us = us_t.pop(b)
            fps = psU.tile([128, 5, 70], F32, tag="pU")
            for mc in range(5):
                w2 = NT[mc]
                nc.tensor.matmul(
                    fps[0:70, 0, 0:70],
                    us[:w2, mc, 0:70],
                    vp[i, vside, mc][:w2, h, 0:70],
                    start=(mc == 0), stop=(mc == 4),
                )
            fb = fpool.tile([70, 70], BF16, tag="fb")
            nc.vector.tensor_copy(out=fb, in_=fps[0:70, 0, 0:70])
            if mh < 5:
                dst = feat8d[mh][i, 0:4900]
            elif i < 4:
                dst = feat5a[i, 0:4900]
            else:
                dst = feat5b[i - 4, 0:4900]
            nc.sync.dma_start(out=dst.rearrange("(d e) -> d e", e=70), in_=fb)

        def emit_gather(ind, outd_, j):
            nc.gpsimd.collective_compute(
                "AllGather",
                OP.bypass,
                replica_groups=[list(range(NCORES))],
                ins=[ind[:]],
                outs=[outd_[:]],
            )

        def emit_proj(src, mh, ncols, rowoff):
            # ncols = gathered item count; rowoff: None = all 64 items in
            # core-major order, else item offset within each core's 8 rows
            ft = ftpool.tile([128, 39, ncols], BF16, tag=f"ft{ncols}")
            nc.sync.dma_start_transpose(out=ft, in_=src[:, 0:PADMH])
            # de rows 4900..4991 of the last chunk are transpose-loaded garbage;
            # zero them (their pwt rows are zero too, but NaN*0 would poison).
            # DMA because engine ops need 32-aligned partition bases.
            nc.sync.dma_start(out=ft[36:128, 38, :], in_=zft[0:92, 0:ncols])
            pw = ftpool.tile([128, 39, OS], BF16, tag="pw")
            nc.sync.dma_start(
                out=pw,
                in_=pwtd[mh * PADMH : (mh + 1) * PADMH, :].rearrange(
                    "(j p) o -> p j o", p=128
                ),
            )
            opsum = psU.tile([128, 5, 70], F32, tag="pU", name="opsum")
            for j in range(39):
                nc.tensor.matmul(
                    opsum[0:ncols, 0, 0:OS],
                    ft[:, j, :], pw[:, j, :],
                    start=(j == 0), stop=(j == 38),
                )
            if rowoff is None:
                nc.vector.tensor_add(oacc, oacc, opsum[0:B, 0, 0:OS])
            else:
                # scattered-row accumulate: engines need 32-aligned partition
                # bases, so copy psum->sbuf aligned then DMA-accumulate the
                # strided row sets into oacc
                osb5 = opool.tile([32, OS], F32, tag="osb5", bufs=2)
                nc.vector.tensor_copy(osb5, opsum[0:32, 0, 0:OS])
                av = oacc.rearrange("(r i) o -> i r o", i=BP)
                sv = osb5.rearrange("(r i) o -> i r o", i=4)
                for io in range(4):
                    nc.gpsimd.dma_start(
                        out=av[rowoff + io], in_=sv[io], accum_op=OP.add
                    )

        # pipeline, nt-tile granular: step s handles exp/Zsum of block b=s-1,
        # QK tiles running 2 ticks ahead (A psum rotation of 2), the U/us/F
        # stages of block s-3 (all inputs a full step old), and the DVE
        # chain of block s-2 emitted first so the DVE queue head never
        # stalls on same-step producers.
        gather_after = {}  # step -> list of (in, out, j)
        for mh in range(5):
            gather_after.setdefault(mh * 8 + 7 + 3, []).append(
                (feat8d[mh], featAG[mh], mh)
            )
        gather_after.setdefault(5 * 8 + 3 + 3, []).append((feat5a, featAG5a, 5))
        proj_after = {}
        for mh in range(5):
            proj_after.setdefault(min(mh * 8 + 17, NB + 2), []).append(mh)

        NG = 5 * NB
        for s in range(NB + 3):
            if s >= 2 and s - 2 < NB:
                emit_dve_chain(s - 2)
            for k in range(5):
                g = 5 * (s - 1) + k
                if 0 <= g < NG:
                    emit_exp_tile(g)
                    emit_zsum_tile(g)
                if 0 <= g + 2 < NG:
                    emit_qk_tile(g + 2)
                if s >= 3:
                    emit_u_group(s - 3, k)
            if s >= 3:
                emit_us(s - 3)
                emit_f(s - 3)
            for g in gather_after.get(s, []):
                emit_gather(*g)
            for mh in proj_after.get(s, []):
                emit_proj(featAG[mh], mh, B, None)

        # tail: gather second half of mh5, then its two half projections
        emit_gather(feat5b, featAG5b, 5)
        emit_proj(featAG5a, 5, B // 2, 0)
        emit_proj(featAG5b, 5, B // 2, 4)
        osb = opool.tile([B, OS], F32, tag="osb")
        nc.vector.tensor_add(osb, oacc, pb_sb)
        nc.vector.tensor_scalar_max(osb, osb, 0.0)
        nc.sync.dma_start(out=outd[:], in_=osb)

    nc.compile()
    return nc


def kernel(x1, x2, ln_w, ln_b, qkv_w, proj_w, proj_b):
    wqk, wv, pos_pad, pwt = _host_prep(ln_w, ln_b, qkv_w, proj_w, proj_b)
    xs1 = np.ascontiguousarray(x1.reshape(B, C, N)).astype(ml_dtypes.bfloat16)
    xs2 = np.ascontiguousarray(x2.reshape(B, C, N)).astype(ml_dtypes.bfloat16)
    nc = _build()
    in_maps = []
    for r in range(NCORES):
        in_maps.append(
            {
                "x1s": xs1[r * BP : (r + 1) * BP],
                "x2s": xs2[r * BP : (r + 1) * BP],
                "wqk": wqk,
                "wv": wv,
                "pos": pos_pad,
                "pwt": np.ascontiguousarray(pwt[:, r * OS : (r + 1) * OS]),
                "pb": np.ascontiguousarray(proj_b[None, r * OS : (r + 1) * OS]).astype(np.float32),
            }
        )
    import os
    import time as _time

    trace = bool(os.environ.get("BASS_TRACE"))
    res = run_bass_kernel_spmd(nc, in_maps, core_ids=list(range(NCORES)), trace=trace)
    nruns = int(os.environ.get("PROF_RUNS", "0"))
    if nruns:
        times = []
        for _ in range(nruns):
            t0 = _time.perf_counter()
            run_bass_kernel_spmd(nc, in_maps, core_ids=list(range(NCORES)))
            times.append(_time.perf_counter() - t0)
        print(f"exec wall times (s): {[f'{t:.3f}' for t in times]}, min={min(times)*1e6:.0f} us")
    if res.exec_time_ns is not None:
        print(f"HW exec time: {res.exec_time_ns} ns")
    if res.instructions_and_trace:
        print("trace path:", res.instructions_and_trace[1])
    out = np.concatenate([res.results[r]["out"] for r in range(NCORES)], axis=1)
    return out.astype(np.float32)


if __name__ == "__main__":
    rng = np.random.default_rng(0)
    ins = {
        "x1": rng.standard_normal((B, C, HG, WG), dtype=np.float32),
        "x2": rng.standard_normal((B, C, HG, WG), dtype=np.float32),
        "ln_w": np.ones(C, np.float32),
        "ln_b": np.zeros(C, np.float32),
        "qkv_w": (rng.standard_normal((3 * F, C)) * C**-0.5).astype(np.float32),
        "proj_w": (rng.standard_normal((512, 6 * 4900)) * (6 * 4900) ** -0.5).astype(np.float32),
        "proj_b": np.zeros(512, np.float32),
    }
    print(kernel(**ins).shape)
